# revision 11
# baseline (speedup 1.0000x reference)
"""Binarized dense layer on 8 Trainium2 NeuronCores.

Computes relu(x @ sign(W) + b) for x,W [4096,4096] f32, b [4096] f32.

Sharding: 2-D grid over (batch M, units N). Each core gets
  xT chunk [K=4096, Mc]   (x rows transposed -> contraction on partitions),
                          cast to bf16 on host (halves the startup DMA;
                          sign(W) is exact in bf16 so only x quantization
                          contributes error, ~7e-4 relative)
  W  chunk [K=4096, Nc]   (column shard, streamed as f32)
  b  chunk [Nc]
producing outT chunk [Nc, Mc] (host transposes back).

Per-core: x resident in SBUF. W streamed, binarized on DVE to
{+0.5,-0.5} bf16 in one fused op (is_ge, -0.5); the factor 2 is folded
into the epilogue scale. PE: bf16 matmuls, stationary = sign(W) tile,
moving = x [128,512], accumulating k=0..31 into all 8 PSUM banks.
Epilogue: single ScalarE op out = Relu(2*psum + b[n]) straight from PSUM,
then DMA out.
"""

import numpy as np
import ml_dtypes

import concourse.bass as bass
import concourse.bacc as bacc
import concourse.mybir as mybir
import concourse.tile as tile
from concourse.bass_utils import run_bass_kernel_spmd

_B, _K, _N = 4096, 4096, 4096
_RM, _CN = 4, 2  # grid: M split x N split
_MC, _NC = _B // _RM, _N // _CN  # per-core M, N

_P = 128
_KT = _K // _P  # 32 k-tiles
_MCH = _MC // 512  # m-chunks of 512
_NT = _NC // _P  # n-tiles
_NT_PER_G = 8 // _MCH  # n-tiles per group so that ntiles*mchunks = 8 banks

_AF = mybir.ActivationFunctionType
_ALU = mybir.AluOpType

_NC_CACHE = None
LAST_EXEC_NS = None
LAST_TRACE = None


def _build():
    nc = bacc.Bacc(
        trn_type="TRN2", target_bir_lowering=False, debug=False, num_devices=8
    )
    f32 = mybir.dt.float32
    bf16 = mybir.dt.bfloat16

    xT_d = nc.dram_tensor("xT", [_K, _MC], bf16, kind="ExternalInput")
    w_d = nc.dram_tensor("w", [_K, _NC], f32, kind="ExternalInput")
    b_d = nc.dram_tensor("b", [_NC], f32, kind="ExternalInput")
    outT_d = nc.dram_tensor("outT", [_NC, _MC], f32, kind="ExternalOutput")

    ngroups = _NT // _NT_PER_G
    wcols = _NT_PER_G * _P  # W columns consumed per k-step

    with tile.TileContext(nc) as tc:
        with (
            tc.tile_pool(name="xres", bufs=1) as xres,
            tc.tile_pool(name="wio", bufs=8) as wio,
            tc.tile_pool(name="wsg", bufs=8) as wsgp,
            tc.tile_pool(name="oio", bufs=8) as oio,
            tc.tile_pool(name="bio", bufs=1) as bio,
            tc.tile_pool(name="psum", bufs=8, space="PSUM") as pp,
        ):
            b_sb = bio.tile([_P, _NT], f32, name="b_sb")

            xs = []
            for k in range(_KT):
                xs.append(xres.tile([_P, _MC], bf16, name=f"xs{k}"))

            # (first n-tile, n-tile count) per group; last group split in two
            # so the final PSUM drain is half as deep.
            gspecs = []
            for g in range(ngroups - 1):
                gspecs.append((g * _NT_PER_G, _NT_PER_G))
            last0 = (ngroups - 1) * _NT_PER_G
            gspecs.append((last0, _NT_PER_G // 2))
            gspecs.append((last0 + _NT_PER_G // 2, _NT_PER_G - _NT_PER_G // 2))

            for gi, (nt0, nnt) in enumerate(gspecs):
                gwcols = nnt * _P
                ps = {}
                for nt in range(nnt):
                    for m in range(_MCH):
                        ps[(nt, m)] = pp.tile([_P, 512], f32, name="ps", tag="ps")
                for k in range(_KT):
                    prime = gi == 0 and k == 0
                    wst = wio.tile([_P, wcols], f32, name="wst", tag="wst")
                    wsg = wsgp.tile([_P, wcols], bf16, name="wsg", tag="wsg")
                    if prime:
                        # fine-grained first transfers: the DMA pipe takes
                        # several us to ramp, so make the first MM's
                        # dependencies as small as possible.
                        nc.sync.dma_start(
                            xs[0][:, 0:512], xT_d[0:_P, 0:512]
                        )
                        for c in range(nnt):
                            nc.sync.dma_start(
                                wst[:, c * _P : (c + 1) * _P],
                                w_d[0:_P, c * _P : (c + 1) * _P],
                            )
                            nc.vector.tensor_scalar(
                                wsg[:, c * _P : (c + 1) * _P],
                                wst[:, c * _P : (c + 1) * _P],
                                0.0, 0.5, _ALU.is_ge, _ALU.subtract,
                            )
                        nc.sync.dma_start(
                            xs[0][:, 512:1024], xT_d[0:_P, 512:1024]
                        )
                    else:
                        if gi == 0:
                            nc.sync.dma_start(
                                xs[k][:], xT_d[k * _P : (k + 1) * _P, :]
                            )
                        nc.sync.dma_start(
                            wst[:, :gwcols],
                            w_d[k * _P : (k + 1) * _P, nt0 * _P : nt0 * _P + gwcols],
                        )
                        # sign(w)/2 in {+0.5,-0.5}; matches where(w>=0,1,-1)/2
                        nc.vector.tensor_scalar(
                            wsg[:, :gwcols], wst[:, :gwcols], 0.0, 0.5,
                            _ALU.is_ge, _ALU.subtract,
                        )
                    loop = (
                        [(nt, m) for m in range(_MCH) for nt in range(nnt)]
                        if prime
                        else [(nt, m) for nt in range(nnt) for m in range(_MCH)]
                    )
                    for nt, m in loop:
                        nc.tensor.matmul(
                            ps[(nt, m)][:],
                            wsg[:, nt * _P : (nt + 1) * _P],
                            xs[k][:, m * 512 : (m + 1) * 512],
                            start=(k == 0),
                            stop=(k == _KT - 1),
                        )
                if gi == 0:
                    nc.sync.dma_start(
                        b_sb[:, :], b_d.rearrange("(o p) -> p o", p=_P)
                    )
                osbs = {}
                for nt in range(nnt):
                    gnt = nt0 + nt
                    for m in range(_MCH):
                        osb = oio.tile([_P, 512], f32, name="osb", tag="osb")
                        osbs[(nt, m)] = osb
                        nc.scalar.activation(
                            osb[:],
                            ps[(nt, m)][:],
                            _AF.Relu,
                            bias=b_sb[:, gnt : gnt + 1],
                            scale=2.0,
                        )
                for nt in range(nnt):
                    gnt = nt0 + nt
                    for m in range(_MCH):
                        nc.scalar.dma_start(
                            outT_d[
                                gnt * _P : (gnt + 1) * _P, m * 512 : (m + 1) * 512
                            ],
                            osbs[(nt, m)][:],
                        )
    nc.compile()
    return nc


def _install_ntff_shim():
    """Provide antenv.axon_hooks (absent in this image) so that
    run_bass_kernel_spmd(trace=True) can NTFF-profile via the axon .so."""
    import sys
    import types
    import ctypes
    import contextlib

    if "antenv.axon_hooks" in sys.modules:
        return
    so_path = "/opt/axon/libaxon_pjrt.so"
    try:
        lib = ctypes.CDLL(so_path)
        lib.axon_start_nrt_profile.argtypes = [
            ctypes.POINTER(ctypes.c_int64),
            ctypes.c_size_t,
        ]
        lib.axon_start_nrt_profile.restype = ctypes.c_int64
        lib.axon_stop_nrt_profile.argtypes = [ctypes.c_char_p]
        lib.axon_stop_nrt_profile.restype = ctypes.c_int64
    except (OSError, AttributeError):
        lib = None

    @contextlib.contextmanager
    def _hook(output_dir, device_ids):
        import jax

        jax.devices()
        if device_ids:
            ids = (ctypes.c_int64 * len(device_ids))(*device_ids)
            rc = lib.axon_start_nrt_profile(ids, len(device_ids))
        else:
            rc = lib.axon_start_nrt_profile(None, 0)
        if rc != 0:
            raise RuntimeError(f"axon_start_nrt_profile rc={rc}")
        try:
            yield
        finally:
            n = lib.axon_stop_nrt_profile(str(output_dir).encode())
            print(f"ntff profile: {n} file(s) written to {output_dir}")

    mod = types.ModuleType("antenv.axon_hooks")
    mod.get_axon_ntff_profile_hook = lambda: (_hook if lib is not None else None)
    mod.set_axon_ntff_profile_hook = lambda h: None
    sys.modules["antenv.axon_hooks"] = mod


def kernel(x: np.ndarray, W: np.ndarray, b: np.ndarray) -> np.ndarray:
    global _NC_CACHE, LAST_EXEC_NS, LAST_TRACE
    import os

    x = np.ascontiguousarray(np.asarray(x, dtype=np.float32))
    W = np.ascontiguousarray(np.asarray(W, dtype=np.float32))
    b = np.ascontiguousarray(np.asarray(b, dtype=np.float32))

    if _NC_CACHE is None:
        _NC_CACHE = _build()
    nc = _NC_CACHE

    xT_chunks = [
        np.ascontiguousarray(x[i * _MC : (i + 1) * _MC, :].T).astype(
            ml_dtypes.bfloat16
        )
        for i in range(_RM)
    ]
    w_chunks = [
        np.ascontiguousarray(W[:, j * _NC : (j + 1) * _NC]) for j in range(_CN)
    ]
    b_chunks = [np.ascontiguousarray(b[j * _NC : (j + 1) * _NC]) for j in range(_CN)]

    in_maps = []
    for core in range(8):
        i, j = core // _CN, core % _CN
        in_maps.append({"xT": xT_chunks[i], "w": w_chunks[j], "b": b_chunks[j]})

    trace = bool(int(os.environ.get("KERNEL_TRACE", "0")))
    if trace:
        _install_ntff_shim()
    res = run_bass_kernel_spmd(
        nc, in_maps, core_ids=list(range(8)), trace=trace
    )
    LAST_EXEC_NS = res.exec_time_ns
    LAST_TRACE = res.instructions_and_trace

    out = np.empty((_B, _N), dtype=np.float32)
    for core in range(8):
        i, j = core // _CN, core % _CN
        out[i * _MC : (i + 1) * _MC, j * _NC : (j + 1) * _NC] = res.results[core][
            "outT"
        ].T
    return out


# revision 12
# speedup vs baseline: 1.0212x; 1.0212x over previous
"""Binarized dense layer on 8 Trainium2 NeuronCores.

Computes relu(x @ sign(W) + b) for x,W [4096,4096] f32, b [4096] f32.

Sharding: 2-D grid over (batch M, units N). Each core gets
  xT chunk [K=4096, Mc]   (x rows transposed -> contraction on partitions),
                          cast to bf16 on host (halves the startup DMA;
                          sign(W) is exact in bf16 so only x quantization
                          contributes error, ~7e-4 relative)
  W  chunk [K=4096, Nc]   (column shard, streamed as f32)
  b  chunk [Nc]
producing outT chunk [Nc, Mc] (host transposes back).

Per-core: x resident in SBUF. W streamed, binarized on DVE to
{+0.5,-0.5} bf16 in one fused op (is_ge, -0.5); the factor 2 is folded
into the epilogue scale. PE: bf16 matmuls, stationary = sign(W) tile,
moving = x [128,512], accumulating k=0..31 into all 8 PSUM banks.
Epilogue: single ScalarE op out = Relu(2*psum + b[n]) straight from PSUM,
then DMA out.
"""

import numpy as np
import ml_dtypes

import concourse.bass as bass
import concourse.bacc as bacc
import concourse.mybir as mybir
import concourse.tile as tile
from concourse.bass_utils import run_bass_kernel_spmd

_B, _K, _N = 4096, 4096, 4096
_RM, _CN = 4, 2  # grid: M split x N split
_MC, _NC = _B // _RM, _N // _CN  # per-core M, N

_P = 128
_KT = _K // _P  # 32 k-tiles
_MCH = _MC // 512  # m-chunks of 512
_NT = _NC // _P  # n-tiles
_NT_PER_G = 8 // _MCH  # n-tiles per group so that ntiles*mchunks = 8 banks

_AF = mybir.ActivationFunctionType
_ALU = mybir.AluOpType

_NC_CACHE = None
LAST_EXEC_NS = None
LAST_TRACE = None


def _build():
    nc = bacc.Bacc(
        trn_type="TRN2", target_bir_lowering=False, debug=False, num_devices=8
    )
    f32 = mybir.dt.float32
    bf16 = mybir.dt.bfloat16

    xT_d = nc.dram_tensor("xT", [_K, _MC], bf16, kind="ExternalInput")
    w_d = nc.dram_tensor("w", [_K, _NC], f32, kind="ExternalInput")
    b_d = nc.dram_tensor("b", [_NC], f32, kind="ExternalInput")
    outT_d = nc.dram_tensor("outT", [_NC, _MC], f32, kind="ExternalOutput")

    ngroups = _NT // _NT_PER_G
    wcols = _NT_PER_G * _P  # W columns consumed per k-step

    with tile.TileContext(nc) as tc:
        with (
            tc.tile_pool(name="xres", bufs=1) as xres,
            tc.tile_pool(name="wio", bufs=8) as wio,
            tc.tile_pool(name="wsg", bufs=8) as wsgp,
            tc.tile_pool(name="oio", bufs=8) as oio,
            tc.tile_pool(name="bio", bufs=1) as bio,
            tc.tile_pool(name="psum", bufs=8, space="PSUM") as pp,
        ):
            b_sb = bio.tile([_P, _NT], f32, name="b_sb")

            xs = []
            for k in range(_KT):
                xs.append(xres.tile([_P, _MC], bf16, name=f"xs{k}"))

            # (first n-tile, n-tile count) per group; last group split in two
            # so the final PSUM drain is half as deep.
            gspecs = []
            for g in range(ngroups - 1):
                gspecs.append((g * _NT_PER_G, _NT_PER_G))
            last0 = (ngroups - 1) * _NT_PER_G
            gspecs.append((last0, _NT_PER_G // 2))
            gspecs.append((last0 + _NT_PER_G // 2, _NT_PER_G - _NT_PER_G // 2))

            for gi, (nt0, nnt) in enumerate(gspecs):
                gwcols = nnt * _P
                ps = {}
                for nt in range(nnt):
                    for m in range(_MCH):
                        ps[(nt, m)] = pp.tile([_P, 512], f32, name="ps", tag="ps")
                for k in range(_KT):
                    if gi == 0:
                        nc.sync.dma_start(xs[k][:], xT_d[k * _P : (k + 1) * _P, :])
                    wst = wio.tile([_P, wcols], f32, name="wst", tag="wst")
                    nc.sync.dma_start(
                        wst[:, :gwcols],
                        w_d[k * _P : (k + 1) * _P, nt0 * _P : nt0 * _P + gwcols],
                    )
                    wsg = wsgp.tile([_P, wcols], bf16, name="wsg", tag="wsg")
                    # sign(w)/2 in {+0.5,-0.5}; matches where(w>=0,1,-1)/2
                    nc.vector.tensor_scalar(
                        wsg[:, :gwcols], wst[:, :gwcols], 0.0, 0.5,
                        _ALU.is_ge, _ALU.subtract,
                    )
                    for nt in range(nnt):
                        for m in range(_MCH):
                            nc.tensor.matmul(
                                ps[(nt, m)][:],
                                wsg[:, nt * _P : (nt + 1) * _P],
                                xs[k][:, m * 512 : (m + 1) * 512],
                                start=(k == 0),
                                stop=(k == _KT - 1),
                            )
                if gi == 0:
                    nc.sync.dma_start(
                        b_sb[:, :], b_d.rearrange("(o p) -> p o", p=_P)
                    )
                last = gi == len(gspecs) - 1
                osbs = {}
                for nt in range(nnt):
                    gnt = nt0 + nt
                    for m in range(_MCH):
                        osb = oio.tile([_P, 512], f32, name="osb", tag="osb")
                        osbs[(nt, m)] = osb
                        nc.scalar.activation(
                            osb[:],
                            ps[(nt, m)][:],
                            _AF.Relu,
                            bias=b_sb[:, gnt : gnt + 1],
                            scale=2.0,
                        )
                        if last:
                            # sync queue is idle by now; store there right
                            # after each activation to shorten the drain
                            nc.sync.dma_start(
                                outT_d[
                                    gnt * _P : (gnt + 1) * _P,
                                    m * 512 : (m + 1) * 512,
                                ],
                                osb[:],
                            )
                if not last:
                    for nt in range(nnt):
                        gnt = nt0 + nt
                        for m in range(_MCH):
                            nc.scalar.dma_start(
                                outT_d[
                                    gnt * _P : (gnt + 1) * _P,
                                    m * 512 : (m + 1) * 512,
                                ],
                                osbs[(nt, m)][:],
                            )
    nc.compile()
    return nc


def _install_ntff_shim():
    """Provide antenv.axon_hooks (absent in this image) so that
    run_bass_kernel_spmd(trace=True) can NTFF-profile via the axon .so."""
    import sys
    import types
    import ctypes
    import contextlib

    if "antenv.axon_hooks" in sys.modules:
        return
    so_path = "/opt/axon/libaxon_pjrt.so"
    try:
        lib = ctypes.CDLL(so_path)
        lib.axon_start_nrt_profile.argtypes = [
            ctypes.POINTER(ctypes.c_int64),
            ctypes.c_size_t,
        ]
        lib.axon_start_nrt_profile.restype = ctypes.c_int64
        lib.axon_stop_nrt_profile.argtypes = [ctypes.c_char_p]
        lib.axon_stop_nrt_profile.restype = ctypes.c_int64
    except (OSError, AttributeError):
        lib = None

    @contextlib.contextmanager
    def _hook(output_dir, device_ids):
        import jax

        jax.devices()
        if device_ids:
            ids = (ctypes.c_int64 * len(device_ids))(*device_ids)
            rc = lib.axon_start_nrt_profile(ids, len(device_ids))
        else:
            rc = lib.axon_start_nrt_profile(None, 0)
        if rc != 0:
            raise RuntimeError(f"axon_start_nrt_profile rc={rc}")
        try:
            yield
        finally:
            n = lib.axon_stop_nrt_profile(str(output_dir).encode())
            print(f"ntff profile: {n} file(s) written to {output_dir}")

    mod = types.ModuleType("antenv.axon_hooks")
    mod.get_axon_ntff_profile_hook = lambda: (_hook if lib is not None else None)
    mod.set_axon_ntff_profile_hook = lambda h: None
    sys.modules["antenv.axon_hooks"] = mod


def kernel(x: np.ndarray, W: np.ndarray, b: np.ndarray) -> np.ndarray:
    global _NC_CACHE, LAST_EXEC_NS, LAST_TRACE
    import os

    x = np.ascontiguousarray(np.asarray(x, dtype=np.float32))
    W = np.ascontiguousarray(np.asarray(W, dtype=np.float32))
    b = np.ascontiguousarray(np.asarray(b, dtype=np.float32))

    if _NC_CACHE is None:
        _NC_CACHE = _build()
    nc = _NC_CACHE

    xT_chunks = [
        np.ascontiguousarray(x[i * _MC : (i + 1) * _MC, :].T).astype(
            ml_dtypes.bfloat16
        )
        for i in range(_RM)
    ]
    w_chunks = [
        np.ascontiguousarray(W[:, j * _NC : (j + 1) * _NC]) for j in range(_CN)
    ]
    b_chunks = [np.ascontiguousarray(b[j * _NC : (j + 1) * _NC]) for j in range(_CN)]

    in_maps = []
    for core in range(8):
        i, j = core // _CN, core % _CN
        in_maps.append({"xT": xT_chunks[i], "w": w_chunks[j], "b": b_chunks[j]})

    trace = bool(int(os.environ.get("KERNEL_TRACE", "0")))
    if trace:
        _install_ntff_shim()
    res = run_bass_kernel_spmd(
        nc, in_maps, core_ids=list(range(8)), trace=trace
    )
    LAST_EXEC_NS = res.exec_time_ns
    LAST_TRACE = res.instructions_and_trace

    out = np.empty((_B, _N), dtype=np.float32)
    for core in range(8):
        i, j = core // _CN, core % _CN
        out[i * _MC : (i + 1) * _MC, j * _NC : (j + 1) * _NC] = res.results[core][
            "outT"
        ].T
    return out
